# revision 22
# baseline (speedup 1.0000x reference)
"""Distributed multi-head attention for 8 trn2 NeuronCores.

Strategy (Ulysses-style head-sharding):
  - Every core receives the full activations pre-transposed/cast host-side:
    xT [C, B*N] bf16. Heads are sharded 2-per-core for QKV + attention
    (Megatron column-sharded QKV weights). Scores are computed TRANSPOSED
    (keys on partitions, queries on free) so the softmaxed probabilities
    feed the P@V matmul directly with no on-chip transposes of P. The
    softmax denominator comes for free from a ones-column appended to V.
  - A per-batch AllToAll (bf16) re-shards the attention output from
    head-sharded to row-sharded; the output projection then runs with the
    full Wproj.T per core plus bias. Core i returns rows
    {256i..256i+256} of each batch; the host reassembles the full output.

Walrus constraint: a fused matmul carries at most ONE semaphore wait; an
explicit ldweights before each accumulation-group start gives
move_matmul_waits_to_ldweights a place to park extra waits.
"""

import sys

for _p in ("/opt/trn_rl_repo", "/opt/pypackages"):
    if _p not in sys.path:
        sys.path.append(_p)

import numpy as np
import ml_dtypes

import concourse.bass as bass
import concourse.mybir as mybir
import concourse.tile as tile
from concourse import bacc
from concourse.bass_utils import run_bass_kernel_spmd

P = 128
CORES = 8
B, N, C = 2, 2048, 1024
H, D = 16, 64
R = B * N          # 4096 total rows
HL = H // CORES    # 2 heads per core
DL = HL * D        # 128 head dims per core
RO = R // CORES    # 512 output rows per core
RB = RO // B       # 256 rows per (core, batch)
NKC = N // P       # 16 key chunks of 128 per batch
NQC = N // 512     # 4 query chunks of 512 per batch
CK = C // P        # 8 contraction chunks of 128
SCALE = D ** -0.5  # 0.125

F32 = mybir.dt.float32
BF16 = mybir.dt.bfloat16

NH = 1            # A2A splits per batch (1 or 2); BLK = per-core rows per split
BLK = N // CORES // NH


def build_nc():
    nc = bacc.Bacc("TRN2", target_bir_lowering=False, debug=False,
                   num_devices=CORES)

    xT_d = nc.declare_dram_parameter("xT", [C, R], BF16, isOutput=False)
    wq_d = nc.declare_dram_parameter("wqT", [C, DL], BF16, isOutput=False)
    wk_d = nc.declare_dram_parameter("wkT", [C, DL], BF16, isOutput=False)
    wv_d = nc.declare_dram_parameter("wvT", [C, DL], BF16, isOutput=False)
    wp_d = nc.declare_dram_parameter("wpT", [C, C], BF16, isOutput=False)
    bp_d = nc.declare_dram_parameter("bproj", [C], F32, isOutput=False)
    out_d = nc.declare_dram_parameter("out", [RO, C], F32, isOutput=True)

    with tile.TileContext(nc) as tc:
        build_kernel(tc, xT_d, wq_d, wk_d, wv_d, wp_d, bp_d, out_d)

    nc.compile()
    return nc


def build_kernel(tc, xT_d, wq_d, wk_d, wv_d, wp_d, bp_d, out_d):
    nc = tc.nc
    EXP = mybir.ActivationFunctionType.Exp

    with (
        tc.tile_pool(name="persist", bufs=1) as persist,
        tc.tile_pool(name="expp", bufs=3) as expp,
        tc.tile_pool(name="small", bufs=4) as small,
        tc.tile_pool(name="ypool", bufs=3) as ypool,
        tc.tile_pool(name="pA", bufs=2, space="PSUM") as pA,
        tc.tile_pool(name="pB", bufs=2, space="PSUM") as pB,
        tc.tile_pool(name="dramp", bufs=1, space="DRAM") as dramp,
    ):
        # ---------------- persistent SBUF tensors ----------------
        xT_sb = persist.tile([P, CK, R], BF16, name="xT_sb")
        wq_sb = persist.tile([P, CK, DL], BF16, name="wq_sb")
        wk_sb = persist.tile([P, CK, DL], BF16, name="wk_sb")
        wv_sb = persist.tile([P, CK, DL], BF16, name="wv_sb")
        wp_sb = persist.tile([P, CK, C], BF16, name="wp_sb")
        bias_sb = persist.tile([P, C], F32, name="bias_sb")
        qT_sb = persist.tile([P, R], BF16, name="qT_sb")
        kT_sb = persist.tile([P, R], BF16, name="kT_sb")
        vT_sb = persist.tile([P, R], BF16, name="vT_sb")
        # vaug[:, idx(b,h,kc), :]: [keys=128, D+1]; col D holds ones
        vaug_sb = persist.tile([P, B * HL * NKC, D + 1], BF16, name="vaug_sb")
        oT0_sb = persist.tile([D, R], BF16, name="oT0_sb")
        oT1_sb = persist.tile([D, R], BF16, name="oT1_sb")
        # gathered layout: [c_in_part, b, half, src_core, BLK rows]
        oTg_sb = persist.tile([P, B, NH, CORES, BLK], BF16, name="oTg_sb")
        warm_sb = persist.tile([P, 512], BF16, name="warm_sb")

        # one A2A per (batch, half-batch): blocks [core, 128 cin, BLK rows]
        a2a_in = [
            dramp.tile([CORES, DL, BLK], BF16, name=f"a2a_in{i}")
            for i in range(NH * B)
        ]
        a2a_out = [
            dramp.tile([CORES, DL, BLK], BF16, name=f"a2a_out{i}")
            for i in range(NH * B)
        ]
        rscr = [dramp.tile([1024], F32, name=f"rscr{i}") for i in range(4)]

        def vidx(b, h, kc):
            return (b * HL + h) * NKC + kc

        # ---------------- constants / input DMAs ----------------
        nc.vector.memset(vaug_sb[:, :, D], 1.0)
        nc.vector.memset(warm_sb, 0.0)

        # PE warmup while input DMAs land: ~18 back-to-back matmuls push the
        # HAM clock gate to 8/8 before real work starts
        wps = pA.tile([P, 1024], F32, tag="big", name="wps")
        for i in range(18):
            nc.tensor.matmul(wps[:, 0:512], lhsT=warm_sb[:, 0:128],
                             rhs=warm_sb, start=(i == 0), stop=(i == 17))

        xT_ap = xT_d.ap().rearrange("(o p) n -> p o n", p=P)
        for rc in range(R // 512):
            sl = slice(rc * 512, (rc + 1) * 512)
            nc.sync.dma_start(out=xT_sb[:, :, sl], in_=xT_ap[:, :, sl])

        nc.sync.dma_start(out=wq_sb,
                          in_=wq_d.ap().rearrange("(o p) d -> p o d", p=P))
        nc.sync.dma_start(out=wk_sb,
                          in_=wk_d.ap().rearrange("(o p) d -> p o d", p=P))
        nc.sync.dma_start(out=wv_sb,
                          in_=wv_d.ap().rearrange("(o p) d -> p o d", p=P))

        bias_bcast = bass.AP(tensor=bp_d, offset=0, ap=[[0, P], [1, C]])
        nc.gpsimd.dma_start(out=bias_sb, in_=bias_bcast)

        # Wproj is only needed in the epilogue — load it last
        nc.sync.dma_start(out=wp_sb,
                          in_=wp_d.ap().rearrange("(o p) c -> p o c", p=P))

        # ---------------- QKV projections ----------------
        for rc in range(R // 512):
            b, rcl = divmod(rc, NQC)
            sl = slice(rc * 512, (rc + 1) * 512)

            for w_sb, dst in (
                (wq_sb, qT_sb),
                (wk_sb, kT_sb),
                (wv_sb, vT_sb),
            ):
                ps = pA.tile([P, 1024], F32, tag="big", name="ps")
                for o in range(CK):
                    if o == 0:
                        nc.tensor.ldweights(w_sb[:, o])
                    nc.tensor.matmul(ps[:, 0:512], lhsT=w_sb[:, o],
                                     rhs=xT_sb[:, o, sl],
                                     start=(o == 0), stop=(o == CK - 1))
                nc.vector.tensor_copy(out=dst[:, sl], in_=ps[:, 0:512])

            # transpose V into natural [keys, d] layout (both heads at once)
            for t in range(4):
                kcol = rc * 512 + t * 128
                kc_b = rcl * 4 + t
                vtr = expp.tile([P, P], BF16, tag="vtr", name="vtr")
                nc.sync.dma_start_transpose(vtr, vT_sb[:, kcol:kcol + P])
                nc.vector.tensor_copy(out=vaug_sb[:, vidx(b, 0, kc_b), 0:D],
                                      in_=vtr[:, 0:D])
                nc.vector.tensor_copy(out=vaug_sb[:, vidx(b, 1, kc_b), 0:D],
                                      in_=vtr[:, D:2 * D])

        # ---------------- attention + A2A per batch ----------------
        def issue_a2a(b, half):
            i = b * NH + half
            base = b * N + half * (N // NH)
            for j in range(CORES):
                csl = slice(base + j * BLK, base + (j + 1) * BLK)
                nc.scalar.dma_start(out=a2a_in[i][j, 0:D, :],
                                    in_=oT0_sb[:, csl])
                nc.scalar.dma_start(out=a2a_in[i][j, D:2 * D, :],
                                    in_=oT1_sb[:, csl])
            nc.gpsimd.collective_compute(
                "AllToAll",
                mybir.AluOpType.bypass,
                replica_groups=[list(range(CORES))],
                ins=[a2a_in[i].opt()],
                outs=[a2a_out[i].opt()],
            )

        for b in range(B):
            for qc in range(NQC):
                qsl = slice(b * N + qc * 512, b * N + (qc + 1) * 512)
                otF = pB.tile([D + 1, 1024], F32, tag="ot", name="otF")

                def s_pair(kc, qsl=qsl, b=b):
                    kst = b * N + kc * P
                    stF = pA.tile([P, 1024], F32, tag="big", name="stF")
                    for h in range(HL):
                        hsl = slice(h * D, (h + 1) * D)
                        if kc == 0:
                            nc.tensor.ldweights(kT_sb[hsl, kst:kst + P],
                                                tile_position=(h * D, 0))
                        nc.tensor.matmul(stF[:, h * 512:(h + 1) * 512],
                                         lhsT=kT_sb[hsl, kst:kst + P],
                                         rhs=qT_sb[hsl, qsl],
                                         start=True, stop=True)
                    return stF

                def exp_pv(kc, stF, otF=otF, b=b):
                    exF = expp.tile([P, 1024], BF16, tag="exp", name="exF")
                    nc.scalar.activation(out=exF, in_=stF, func=EXP,
                                         scale=SCALE)
                    for h in range(HL):
                        if kc == 0:
                            nc.tensor.ldweights(vaug_sb[:, vidx(b, h, kc), :])
                        nc.tensor.matmul(otF[:, h * 512:(h + 1) * 512],
                                         lhsT=vaug_sb[:, vidx(b, h, kc), :],
                                         rhs=exF[:, h * 512:(h + 1) * 512],
                                         start=(kc == 0), stop=(kc == NKC - 1))

                # software pipeline: scores one kc ahead of exp+PV
                prev = s_pair(0)
                for kc in range(1, NKC):
                    cur = s_pair(kc)
                    exp_pv(kc - 1, prev)
                    prev = cur
                exp_pv(NKC - 1, prev)

                # normalize: fast reciprocal of the denominator row,
                # partition-broadcast, then scale the numerators
                denom = small.tile([1, 1024], F32, tag="denom", name="denom")
                nc.vector.tensor_copy(out=denom, in_=otF[D:D + 1, :])
                recip = small.tile([1, 1024], F32, tag="recip", name="recip")
                nc.vector.reciprocal_approx_fast(out=recip, in_=denom)
                rbc = expp.tile([D, 1024], F32, tag="rbc", name="rbc")
                nc.gpsimd.partition_broadcast(rbc, recip)
                nc.vector.tensor_mul(out=oT0_sb[:, qsl], in0=otF[0:D, 0:512],
                                     in1=rbc[:, 0:512])
                nc.vector.tensor_mul(out=oT1_sb[:, qsl], in0=otF[0:D, 512:],
                                     in1=rbc[:, 512:])

                if NH == 2 and qc == 1:
                    issue_a2a(b, 0)
            issue_a2a(b, NH - 1)

        # ---------------- gather + output projection ----------------
        for b in range(B):
            for h2 in range(NH):
                i = b * NH + h2
                nc.sync.dma_start(out=oTg_sb[:, b, h2],
                                  in_=a2a_out[i].rearrange("k p r -> p k r"))
                for r2 in range(BLK // P):
                    rsl = slice(r2 * P, (r2 + 1) * P)
                    for oc in range(C // 512):
                        osl = slice(oc * 512, (oc + 1) * 512)
                        psy = pA.tile([P, 1024], F32, tag="big", name="psy")
                        for o in range(CK):
                            if o == 0:
                                nc.tensor.ldweights(oTg_sb[:, b, h2, o, rsl])
                            nc.tensor.matmul(psy[:, 0:512],
                                             lhsT=oTg_sb[:, b, h2, o, rsl],
                                             rhs=wp_sb[:, o, osl],
                                             start=(o == 0), stop=(o == CK - 1))
                        y_sb = ypool.tile([P, 512], F32, tag="y", name="y_sb")
                        nc.vector.tensor_add(out=y_sb, in0=psy[:, 0:512],
                                             in1=bias_sb[:, osl])
                        nc.sync.dma_start(
                            out=out_d.ap()[i * BLK + r2 * P:
                                           i * BLK + (r2 + 1) * P, osl],
                            in_=y_sb)


_CACHE = {}


def _get_nc():
    if "nc" not in _CACHE:
        _CACHE["nc"] = build_nc()
    return _CACHE["nc"]


def make_in_maps(x, Wq, Wk, Wv, Wproj, bproj):
    bf = ml_dtypes.bfloat16
    x = np.asarray(x, dtype=np.float32).reshape(R, C)
    xT = np.ascontiguousarray(x.T).astype(bf)
    wpT = np.ascontiguousarray(np.asarray(Wproj, np.float32).T).astype(bf)
    bp = np.ascontiguousarray(np.asarray(bproj, np.float32))
    in_maps = []
    for i in range(CORES):
        hs = slice(DL * i, DL * (i + 1))
        in_maps.append({
            "xT": xT,
            "wqT": np.ascontiguousarray(np.asarray(Wq, np.float32)[hs].T).astype(bf),
            "wkT": np.ascontiguousarray(np.asarray(Wk, np.float32)[hs].T).astype(bf),
            "wvT": np.ascontiguousarray(np.asarray(Wv, np.float32)[hs].T).astype(bf),
            "wpT": wpT,
            "bproj": bp,
        })
    return in_maps


def assemble_out(results):
    # core i's output rows: [(b, half)] chunks of BLK, owning global rows
    # b*N + half*(N//NH) + BLK*i .. +BLK
    y = np.zeros((R, C), np.float32)
    for i in range(CORES):
        o = results[i]["out"]
        for b in range(B):
            for h2 in range(NH):
                g = b * N + h2 * (N // NH) + BLK * i
                c = (b * NH + h2) * BLK
                y[g:g + BLK] = o[c:c + BLK]
    return y.reshape(B, N, C)


def kernel(x, Wq, Wk, Wv, Wproj, bproj):
    nc = _get_nc()
    in_maps = make_in_maps(x, Wq, Wk, Wv, Wproj, bproj)
    res = run_bass_kernel_spmd(nc, in_maps, core_ids=list(range(CORES)))
    return assemble_out(res.results)
